# revision 42
# baseline (speedup 1.0000x reference)
"""Trainium2 Bass kernel for MiniMax softmax attention (T=4096, H=4096, 32 q heads,
8 kv heads, head_dim=128, partial neox RoPE, causal softmax, o_proj).

Sharding: tensor-parallel over heads across 8 NeuronCores. Core c computes q heads
4c..4c+3 (= kv-head group c): qkv^T projection -> RoPE -> causal attention ->
partial o_proj with its w_o row-block. Host sums the 8 partial outputs.

v2 design (vs v1 baseline at ~1210us):
  - all matmul operands bf16 (PSUM accumulates fp32); halves DMA + enables DVE 2x
  - o_proj fused per t-chunk (no DRAM spill of attention outputs)
  - attention kt-major in 2-head passes; softmax denominator ones-matmuls packed
    2-at-a-time into PE column groups via tile_position (M=1 matmuls cost full
    stream time otherwise: 143us of PE in v1)
  - lag-1 software pipelining: AV/denominator matmuls for key-tile kt are emitted
    after the scores matmuls of kt+1 so the tensor FIFO never blocks on ACT exp
  - host pre-tiles every DRAM operand into [128, ...] partition-major contiguous
    blocks for wide DMA lines
PSUM budget: qkv/o_proj cycle pool 2 + scores 2 + AV 2 + denom 2 = 8 banks.
"""
import numpy as np

DEBUG = False

T = 4096
HIDDEN = 4096
NH = 32
NKV = 8
HD = 128
RD = 64
HALF = 32
ROPE_BASE = 10000000.0
NC_CORES = 8
HPC = NH // NC_CORES      # 4 q heads per core
QC = 512                  # t-chunk (query chunk)
NTC = T // QC             # 8 t-chunks
NKO = 32                  # hidden contraction tiles of 128
NJ = HPC + 2              # 6 qkv output tiles of 128 per core (q0..q3, k, v)

_CACHE = {}


def _build_nc():
    import concourse.mybir as mybir
    import concourse.tile as tile
    from concourse import bacc

    F32 = mybir.dt.float32
    BF16 = mybir.dt.bfloat16
    EXP = mybir.ActivationFunctionType.Exp

    nc = bacc.Bacc()
    hidden_p = nc.dram_tensor("hidden_p", [NTC, 128, NKO, QC], BF16, kind="ExternalInput")
    w_prep = nc.dram_tensor("w_prep", [128, NKO, NJ * HD], BF16, kind="ExternalInput")
    wo_prep = nc.dram_tensor("wo_prep", [128, HPC, HIDDEN], BF16, kind="ExternalInput")
    cs_a = nc.dram_tensor("cs_a", [RD, T], BF16, kind="ExternalInput")   # [cos;sin]
    cs_b = nc.dram_tensor("cs_b", [RD, T], BF16, kind="ExternalInput")   # [sin;cos]
    dmask = nc.dram_tensor("dmask", [128, 896], BF16, kind="ExternalInput")
    # out_p[tl, oc] = rows tl*128..tl*128+127, cols oc*512..oc*512+511
    out_p = nc.dram_tensor("out_p", [T // 128, HIDDEN // QC, 128, QC], BF16,
                           kind="ExternalOutput")
    if DEBUG:
        dbg_q = nc.dram_tensor("dbg_q", [NTC, 128, HPC, QC], BF16, kind="ExternalOutput")
        dbg_k = nc.dram_tensor("dbg_k", [NTC, 128, QC], BF16, kind="ExternalOutput")
        dbg_v = nc.dram_tensor("dbg_v", [NTC, 128, 4, 128], BF16, kind="ExternalOutput")
        dbg_dn = nc.dram_tensor("dbg_dn", [NTC, 2, 2, QC], F32, kind="ExternalOutput")
        dbg_ao = nc.dram_tensor("dbg_ao", [NTC, 128, HPC, QC], BF16, kind="ExternalOutput")

    with tile.TileContext(nc) as tc:
        with (
            tc.tile_pool(name="const", bufs=1) as const,
            tc.tile_pool(name="w", bufs=1) as wp,
            tc.tile_pool(name="kv", bufs=1) as kvp,
            tc.tile_pool(name="ht", bufs=3) as htp,
            tc.tile_pool(name="qt", bufs=1) as qtp,
            tc.tile_pool(name="rope", bufs=1) as ropep,
            tc.tile_pool(name="vt", bufs=1) as vtp,
            tc.tile_pool(name="ex", bufs=12) as exp_pool,
            tc.tile_pool(name="exq", bufs=2) as exqp,
            tc.tile_pool(name="ao", bufs=2) as aop,
            tc.tile_pool(name="nrm", bufs=2) as nrmp,
            tc.tile_pool(name="ob", bufs=4) as obp,
            tc.tile_pool(name="cyc", bufs=2, space="PSUM") as cycp,
            tc.tile_pool(name="ssp", bufs=2, space="PSUM") as ssp,
            tc.tile_pool(name="avp", bufs=2, space="PSUM") as avp,
            tc.tile_pool(name="dnp", bufs=2, space="PSUM") as dnp,
        ):
            # ---- constants
            csa_sb = const.tile([RD, T], BF16, name="csa", tag="csa")
            csb_sb = const.tile([RD, T], BF16, name="csb", tag="csb")
            dmask_sb = const.tile([128, 896], BF16, name="dmask", tag="dmask")
            # [128,128] ones: M=128 denominator matmuls cost the same as M=1 but
            # land the denominator pre-broadcast across all psum partitions
            ones_sb = const.tile([128, 128], BF16, name="ones", tag="ones")
            ones_f = const.tile([128, 128], F32, name="ones_f", tag="ones_f")
            nc.gpsimd.memset(ones_f[:], 1.0)
            nc.vector.tensor_copy(ones_sb[:], ones_f[:])
            # HAM warmup: keep the PE busy while the first weight/ht DMAs land
            for _ in range(32):
                wps = cycp.tile([128, 128], F32, name="wps", tag="cyc")
                nc.tensor.matmul(wps[:], ones_sb[:], ones_sb[:], start=True, stop=True)
            # ---- ht halves: [128, 16, 512] each
            def load_ht_half(c, half, pieces=1):
                # pieces>1: finer subtile gating so the first matmuls start early
                htt = htp.tile([128, NKO // 2, QC], BF16, name="ht", tag="ht")
                np_ = 16 // pieces
                for pi in range(pieces):
                    nc.sync.dma_start(
                        htt[:, pi * np_:(pi + 1) * np_, :],
                        hidden_p[c][:, half * 16 + pi * np_:half * 16 + (pi + 1) * np_, :],
                    )
                return htt

            # Startup DMAs on two parallel HWDGE channels: hidden on sync (SP),
            # weights on scalar (ACT DGE — idle before compute), constants on
            # the gpsimd SWDGE. Small first pieces gate the first qkv round.
            w_sb = wp.tile([128, NKO, NJ * HD], BF16, name="w_sb")
            ht0_a = htp.tile([128, NKO // 2, QC], BF16, name="ht", tag="ht")
            for lo, hi in ((0, 1), (1, 2), (2, 4), (4, 8), (8, 16)):
                nc.sync.dma_start(ht0_a[:, lo:hi, :], hidden_p[0][:, lo:hi, :])
            for lo, hi in ((0, 1), (1, 2), (2, 4), (4, 8), (8, 16), (16, 32)):
                nc.scalar.dma_start(w_sb[:, lo:hi, :], w_prep[:, lo:hi, :])
            nc.gpsimd.dma_start(csa_sb[:], cs_a[:])
            nc.gpsimd.dma_start(csb_sb[:], cs_b[:])
            nc.gpsimd.dma_start(dmask_sb[:], dmask[:])
            ht0_b = load_ht_half(0, 1, pieces=2)
            wo_sb = wp.tile([128, HPC, HIDDEN], BF16, name="wo_sb")
            for wi in range(4):
                nc.scalar.dma_start(wo_sb[:, wi, :], wo_prep[:, wi, :])
            ht_halves = [ht0_a, ht0_b]

            # ---- persistent K^T / V tiles (whole sequence, bf16)
            kT_tiles = []
            v_tiles = []
            for i in range(NTC):
                kt_i = kvp.tile([128, QC], BF16, name=f"kT{i}", tag=f"kT{i}")
                v_i = kvp.tile([128, 4, 128], BF16, name=f"v{i}", tag=f"v{i}")
                kT_tiles.append(kt_i)
                v_tiles.append(v_i)

            def rope6(dst):
                # neox partial rope on dst[0:64, :]: x1' = x1*c - x2*s ; x2' = x2*c + x1*s
                x1, x2 = dst[:HALF, :], dst[HALF:RD, :]
                tsl = slice(None)
                t1 = ropep.tile([HALF, QC], BF16, name="r1", tag="r1")
                t2 = ropep.tile([HALF, QC], BF16, name="r2", tag="r2")
                t3 = ropep.tile([HALF, QC], BF16, name="r3", tag="r3")
                t4 = ropep.tile([HALF, QC], BF16, name="r4", tag="r4")
                nc.vector.tensor_mul(t1[:], x1, csa_c[:HALF, :])   # x1*cos
                nc.vector.tensor_mul(t4[:], x1, csb_c[:HALF, :])   # x1*sin
                nc.vector.tensor_mul(t2[:], x2, csa_c[HALF:, :])   # x2*sin
                nc.vector.tensor_sub(x1, t1[:], t2[:])
                nc.vector.tensor_mul(t3[:], x2, csb_c[HALF:, :])   # x2*cos
                nc.vector.tensor_add(x2, t3[:], t4[:])

            def make_oproj_emitters(tci, ao):
                # one closure per (ts, oc) psum group of chunk tci's o_proj.
                # Evictions go to DVE when consumed during a long attention
                # phase (ACT is exp-saturated there), else to ACT.
                ems = []
                for ts in range(QC // 128):
                    for oc in range(HIDDEN // QC):
                        def em(ts=ts, oc=oc, tci=tci, ao=ao):
                            tl = tci * 4 + ts
                            po = cycp.tile([128, QC], F32, name="po", tag="cyc")
                            for h in range(HPC):
                                nc.tensor.matmul(
                                    po[:],
                                    ao[:, h, ts * 128:(ts + 1) * 128],
                                    wo_sb[:, h, oc * QC:(oc + 1) * QC],
                                    start=(h == 0), stop=(h == HPC - 1),
                                )
                            ob = obp.tile([128, QC], BF16, name="ob", tag="ob")
                            if oc % 2 == 0:
                                nc.scalar.copy(ob[:], po[:])
                            else:
                                nc.vector.tensor_copy(ob[:], po[:])
                            nc.sync.dma_start(out_p[tl, oc], ob[:])
                        ems.append(em)
                return ems

            op_ems = []   # deferred o_proj of the previous chunk
            for tci in range(NTC):
                csa_c = csa_sb[:, tci * QC:(tci + 1) * QC]
                csb_c = csb_sb[:, tci * QC:(tci + 1) * QC]

                # ================= phase 1: qkv^T, j-pairs with ko-inner =========
                qcur = qtp.tile([128, HPC, QC], BF16, name="qcur", tag="qt")
                vt = vtp.tile([128, QC], BF16, name="vt", tag="vt")
                ha, hb = ht_halves

                def qkv_round(ja, jb):
                    ps_a = cycp.tile([128, QC], F32, name="psa", tag="cyc")
                    ps_b = cycp.tile([128, QC], F32, name="psb", tag="cyc")
                    for ko in range(NKO):
                        htk = (ha if ko < 16 else hb)[:, ko % 16, :]
                        nc.tensor.matmul(
                            ps_a[:], w_sb[:, ko, ja * HD:(ja + 1) * HD], htk,
                            start=(ko == 0), stop=(ko == NKO - 1),
                        )
                        nc.tensor.matmul(
                            ps_b[:], w_sb[:, ko, jb * HD:(jb + 1) * HD], htk,
                            start=(ko == 0), stop=(ko == NKO - 1),
                        )
                    # evict each bank via ACT+DVE half-copies in parallel so the
                    # next round's matmuls get their psum banks back sooner
                    for idx, (j, ps) in enumerate(((ja, ps_a), (jb, ps_b))):
                        if j == HPC + 1:
                            dst = vt[:]
                        elif j == HPC:
                            dst = kT_tiles[tci][:]
                        else:
                            dst = qcur[:, j, :]
                        lo, hi = slice(0, QC // 2), slice(QC // 2, QC)
                        a, b = (lo, hi) if idx == 0 else (hi, lo)
                        nc.scalar.copy(dst[:, a], ps[:, a])
                        nc.vector.tensor_copy(dst[:, b], ps[:, b])
                        if j == HPC:
                            rope6(kT_tiles[tci])
                        elif j != HPC + 1:
                            rope6(qcur[:, j, :])

                qkv_round(HPC + 1, HPC)   # v, k first
                qkv_round(0, 1)
                qkv_round(2, 3)

                # prefetch next chunk's hidden (half A now, half B mid-attention)
                if tci + 1 < NTC:
                    ht_next_a = load_ht_half(tci + 1, 0)

                # v transpose via the DMA crossbar (no PE/psum involvement)
                nc.sync.dma_start_transpose(v_tiles[tci][:], vt[:])

                # ================= phase 2: attention, 2-head passes, kt-major ===
                # denominators via quad-grouped ex sums (DVE) + one ones-matmul
                # per quad; o_proj of the previous chunk interleaved as filler.
                nkt = 4 * tci + 4
                ngrp = nkt // 4
                kt_steps = 2 * nkt
                step = [0]
                op_i = [0]

                def maybe_fill():
                    step[0] += 1
                    want = (len(op_ems) * step[0]) // kt_steps
                    while op_i[0] < want:
                        op_ems[op_i[0]]()
                        op_i[0] += 1

                ao = aop.tile([128, HPC, QC], BF16, name="ao", tag="ao")

                def attn_pass(pas, qcur=qcur, ao=ao, tci=tci, nkt=nkt, ngrp=ngrp,
                              maybe_fill=maybe_fill):
                    h0 = 2 * pas
                    av0 = avp.tile([128, QC], F32, name="av0", tag="av")
                    av1 = avp.tile([128, QC], F32, name="av1", tag="av")
                    avs = (av0, av1)
                    dn0 = dnp.tile([128, QC], F32, name="dn0", tag="dn")
                    dn1 = dnp.tile([128, QC], F32, name="dn1", tag="dn")
                    dns = (dn0, dn1)

                    def emit_av(pkt, pexs):
                        _po = pkt - 4 * tci
                        pqs = slice(0 if _po < 0 else min(_po * 128, QC - 256), QC)
                        for hh in range(2):
                            nc.tensor.matmul(
                                avs[hh][:, pqs], v_tiles[pkt >> 2][:, pkt & 3, :],
                                pexs[hh][:, pqs],
                                start=(pkt == 0), stop=(pkt == nkt - 1),
                            )

                    def emit_dn(gi, qtiles):
                        for hh in range(2):
                            nc.tensor.matmul(
                                dns[hh][:], ones_sb[:], qtiles[hh][:],
                                start=(gi == 0), stop=(gi == ngrp - 1),
                            )

                    def emit_quad(grp_ex):
                        qtiles = []
                        for hh in range(2):
                            a, b, c, d = grp_ex[hh]
                            s1 = exqp.tile([128, QC], BF16, name="exq1", tag="exq1")
                            s2 = exqp.tile([128, QC], BF16, name="exq2", tag="exq2")
                            nc.vector.tensor_add(s1[:], a[:], b[:])
                            nc.vector.tensor_add(s2[:], c[:], d[:])
                            nc.vector.tensor_add(s1[:], s1[:], s2[:])
                            qtiles.append(s1)
                        return qtiles

                    pend_av = []
                    grp_ex = [[], []]
                    pend_dn = []
                    for kt in range(nkt):
                        _o = kt - 4 * tci
                        qoff = 0 if _o < 0 else min(_o * 128, QC - 256)
                        qs = slice(qoff, QC)
                        kT_l = kT_tiles[kt >> 2][:, (kt & 3) * 128:((kt & 3) + 1) * 128]
                        exs = []
                        for hh in range(2):
                            ss = ssp.tile([128, QC], F32, name="ss", tag="ss")
                            nc.tensor.matmul(
                                ss[:, qs], kT_l, qcur[:, h0 + hh, qs],
                                start=True, stop=True,
                            )
                            # full-width exp: [0:qoff) holds stale-but-finite
                            # scores; the causal mask below zeroes that region.
                            ex = exp_pool.tile([128, QC], BF16, name="ex", tag="ex")
                            nc.scalar.activation(ex[:], ss[:], EXP)
                            if _o >= 0:
                                _off = _o * 128
                                nc.vector.tensor_mul(
                                    ex[:], ex[:], dmask_sb[:, 384 - _off:896 - _off],
                                )
                            exs.append(ex)
                            grp_ex[hh].append(ex)
                        if len(grp_ex[0]) == 4:
                            pend_dn.append((kt // 4, emit_quad(grp_ex)))
                            grp_ex = [[], []]
                        pend_av.append((kt, exs))
                        if len(pend_av) > 2:      # lag-2: DVE mask/quad slack
                            emit_av(*pend_av.pop(0))
                        while pend_dn and kt >= 4 * (pend_dn[0][0] + 1) + 1:
                            emit_dn(*pend_dn.pop(0))
                        maybe_fill()
                    for pa in pend_av:
                        emit_av(*pa)
                    for g in pend_dn:
                        emit_dn(*g)
                    # normalize + evict: ao[:,h,:] = av * (1/denom); denom is
                    # already replicated across partitions by the ones-matmul
                    for hh in range(2):
                        rd_sb = nrmp.tile([128, QC], F32, name="rd", tag="rd")
                        nc.vector.reciprocal_approx_fast(rd_sb[:], dns[hh][:])
                        nc.vector.tensor_mul(ao[:, h0 + hh, :], avs[hh][:], rd_sb[:])
                        if DEBUG:
                            dnc = nrmp.tile([1, QC], F32, name="dnc", tag="dnc")
                            nc.scalar.copy(dnc[:], dns[hh][0:1, :])
                            nc.sync.dma_start(dbg_dn[tci, pas, hh], dnc[:])

                attn_pass(0)
                if tci + 1 < NTC:
                    ht_next_b = load_ht_half(tci + 1, 1)
                attn_pass(1)
                # flush any leftover o_proj of the previous chunk
                while op_i[0] < len(op_ems):
                    op_ems[op_i[0]]()
                    op_i[0] += 1
                op_ems = make_oproj_emitters(tci, ao)
                if DEBUG:
                    nc.sync.dma_start(dbg_q[tci], qcur[:])
                    nc.sync.dma_start(dbg_k[tci], kT_tiles[tci][:])
                    nc.sync.dma_start(dbg_v[tci], v_tiles[tci][:])
                    nc.sync.dma_start(dbg_ao[tci], ao[:])

                if tci + 1 < NTC:
                    ht_halves = [ht_next_a, ht_next_b]

            # o_proj of the final chunk
            for em in op_ems:
                em()
    nc.compile()
    return nc


def _host_prep(positions, hidden_states, w_qkv, w_o):
    import ml_dtypes
    BF = ml_dtypes.bfloat16

    positions = np.asarray(positions)
    hidden_states = np.asarray(hidden_states, dtype=np.float32)
    w_qkv = np.asarray(w_qkv, dtype=np.float32)
    w_o = np.asarray(w_o, dtype=np.float32)

    # hidden_p[c, p, ko, t] = hidden[c*QC + t, ko*128 + p]
    hidden_p = np.ascontiguousarray(
        hidden_states.reshape(NTC, QC, NKO, 128).transpose(0, 3, 2, 1).astype(BF)
    )

    pos = positions.astype(np.float32)
    r = np.arange(0, RD, 2, dtype=np.float32) / np.float32(RD)
    inv_freq = (np.float32(1.0) / (np.float32(ROPE_BASE) ** r)).astype(np.float32)
    ang = pos[:, None] * inv_freq[None, :]
    cos_t = np.cos(ang).astype(np.float32).T       # [32, T]
    sin_t = np.sin(ang).astype(np.float32).T
    cs_a = np.ascontiguousarray(np.concatenate([cos_t, sin_t], 0).astype(BF))
    cs_b = np.ascontiguousarray(np.concatenate([sin_t, cos_t], 0).astype(BF))

    p = np.arange(128, dtype=np.int64)[:, None]
    x = np.arange(896, dtype=np.int64)[None, :]
    dmask = np.ascontiguousarray((x >= p + 384).astype(BF))  # [128, 896]

    scale = np.float32(HD ** -0.5)
    q_size = NH * HD
    kv_size = NKV * HD
    in_maps = []
    for c in range(NC_CORES):
        wq = w_qkv[:, c * HPC * HD:(c + 1) * HPC * HD] * scale
        wk = w_qkv[:, q_size + c * HD:q_size + (c + 1) * HD]
        wv = w_qkv[:, q_size + kv_size + c * HD:q_size + kv_size + (c + 1) * HD]
        w_cat = np.concatenate([wq, wk, wv], axis=1)          # [4096, 768]
        # w_prep[p, ko, j] = w_cat[ko*128 + p, j]
        w_prep = np.ascontiguousarray(
            w_cat.reshape(NKO, 128, NJ * HD).transpose(1, 0, 2).astype(BF)
        )
        # wo_prep[d, h, o] = w_o[(c*HPC + h)*128 + d, o]
        wo_blk = w_o[c * HPC * HD:(c + 1) * HPC * HD, :]
        wo_prep = np.ascontiguousarray(
            wo_blk.reshape(HPC, 128, HIDDEN).transpose(1, 0, 2).astype(BF)
        )
        in_maps.append(
            {
                "hidden_p": hidden_p,
                "w_prep": w_prep,
                "wo_prep": wo_prep,
                "cs_a": cs_a,
                "cs_b": cs_b,
                "dmask": dmask,
            }
        )
    return in_maps


def kernel(positions, hidden_states, w_qkv, w_o, _trace=False, _trace_kw=None):
    from concourse.bass_utils import run_bass_kernel_spmd

    key = f"nc_dbg{DEBUG}"
    if key not in _CACHE:
        _CACHE[key] = _build_nc()
    nc = _CACHE[key]

    in_maps = _host_prep(positions, hidden_states, w_qkv, w_o)
    kw = dict(_trace_kw or {})
    res = run_bass_kernel_spmd(
        nc, in_maps, list(range(NC_CORES)), trace=_trace, **kw
    )
    out = np.zeros((T, HIDDEN), np.float32)
    for c in range(NC_CORES):
        o = np.asarray(res.results[c]["out_p"]).astype(np.float32)
        # [32 tl, 8 oc, 128, 512] -> [4096, 4096]
        out += o.transpose(0, 2, 1, 3).reshape(T, HIDDEN)
    if _trace:
        _CACHE["last_exec_time_ns"] = res.exec_time_ns
        _CACHE["last_results"] = res
    return out


# revision 43
# speedup vs baseline: 1.0104x; 1.0104x over previous
"""Trainium2 Bass kernel for MiniMax softmax attention (T=4096, H=4096, 32 q heads,
8 kv heads, head_dim=128, partial neox RoPE, causal softmax, o_proj).

Sharding: tensor-parallel over heads across 8 NeuronCores. Core c computes q heads
4c..4c+3 (= kv-head group c): qkv^T projection -> RoPE -> causal attention ->
partial o_proj with its w_o row-block. Host sums the 8 partial outputs.

v2 design (vs v1 baseline at ~1210us):
  - all matmul operands bf16 (PSUM accumulates fp32); halves DMA + enables DVE 2x
  - o_proj fused per t-chunk (no DRAM spill of attention outputs)
  - attention kt-major in 2-head passes; softmax denominator ones-matmuls packed
    2-at-a-time into PE column groups via tile_position (M=1 matmuls cost full
    stream time otherwise: 143us of PE in v1)
  - lag-1 software pipelining: AV/denominator matmuls for key-tile kt are emitted
    after the scores matmuls of kt+1 so the tensor FIFO never blocks on ACT exp
  - host pre-tiles every DRAM operand into [128, ...] partition-major contiguous
    blocks for wide DMA lines
PSUM budget: qkv/o_proj cycle pool 2 + scores 2 + AV 2 + denom 2 = 8 banks.
"""
import numpy as np

DEBUG = False

T = 4096
HIDDEN = 4096
NH = 32
NKV = 8
HD = 128
RD = 64
HALF = 32
ROPE_BASE = 10000000.0
NC_CORES = 8
HPC = NH // NC_CORES      # 4 q heads per core
QC = 512                  # t-chunk (query chunk)
NTC = T // QC             # 8 t-chunks
NKO = 32                  # hidden contraction tiles of 128
NJ = HPC + 2              # 6 qkv output tiles of 128 per core (q0..q3, k, v)

_CACHE = {}


def _build_nc():
    import concourse.mybir as mybir
    import concourse.tile as tile
    from concourse import bacc

    F32 = mybir.dt.float32
    BF16 = mybir.dt.bfloat16
    EXP = mybir.ActivationFunctionType.Exp

    nc = bacc.Bacc()
    hidden_p = nc.dram_tensor("hidden_p", [NTC, 128, NKO, QC], BF16, kind="ExternalInput")
    w_prep = nc.dram_tensor("w_prep", [128, NKO, NJ * HD], BF16, kind="ExternalInput")
    wo_prep = nc.dram_tensor("wo_prep", [128, HPC, HIDDEN], BF16, kind="ExternalInput")
    cs_a = nc.dram_tensor("cs_a", [RD, T], BF16, kind="ExternalInput")   # [cos;sin]
    cs_b = nc.dram_tensor("cs_b", [RD, T], BF16, kind="ExternalInput")   # [sin;cos]
    dmask = nc.dram_tensor("dmask", [128, 896], BF16, kind="ExternalInput")
    # out_p[tl, oc] = rows tl*128..tl*128+127, cols oc*512..oc*512+511
    out_p = nc.dram_tensor("out_p", [T // 128, HIDDEN // QC, 128, QC], BF16,
                           kind="ExternalOutput")
    if DEBUG:
        dbg_q = nc.dram_tensor("dbg_q", [NTC, 128, HPC, QC], BF16, kind="ExternalOutput")
        dbg_k = nc.dram_tensor("dbg_k", [NTC, 128, QC], BF16, kind="ExternalOutput")
        dbg_v = nc.dram_tensor("dbg_v", [NTC, 128, 4, 128], BF16, kind="ExternalOutput")
        dbg_dn = nc.dram_tensor("dbg_dn", [NTC, 2, 2, QC], F32, kind="ExternalOutput")
        dbg_ao = nc.dram_tensor("dbg_ao", [NTC, 128, HPC, QC], BF16, kind="ExternalOutput")

    with tile.TileContext(nc) as tc:
        with (
            tc.tile_pool(name="const", bufs=1) as const,
            tc.tile_pool(name="w", bufs=1) as wp,
            tc.tile_pool(name="kv", bufs=1) as kvp,
            tc.tile_pool(name="ht", bufs=3) as htp,
            tc.tile_pool(name="qt", bufs=1) as qtp,
            tc.tile_pool(name="rope", bufs=1) as ropep,
            tc.tile_pool(name="vt", bufs=1) as vtp,
            tc.tile_pool(name="ex", bufs=12) as exp_pool,
            tc.tile_pool(name="exq", bufs=2) as exqp,
            tc.tile_pool(name="ao", bufs=2) as aop,
            tc.tile_pool(name="nrm", bufs=2) as nrmp,
            tc.tile_pool(name="ob", bufs=4) as obp,
            tc.tile_pool(name="cyc", bufs=2, space="PSUM") as cycp,
            tc.tile_pool(name="ssp", bufs=2, space="PSUM") as ssp,
            tc.tile_pool(name="avp", bufs=2, space="PSUM") as avp,
            tc.tile_pool(name="dnp", bufs=2, space="PSUM") as dnp,
        ):
            # ---- constants
            csa_sb = const.tile([RD, T], BF16, name="csa", tag="csa")
            csb_sb = const.tile([RD, T], BF16, name="csb", tag="csb")
            dmask_sb = const.tile([128, 896], BF16, name="dmask", tag="dmask")
            # [128,128] ones: M=128 denominator matmuls cost the same as M=1 but
            # land the denominator pre-broadcast across all psum partitions
            ones_sb = const.tile([128, 128], BF16, name="ones", tag="ones")
            ones_f = const.tile([128, 128], F32, name="ones_f", tag="ones_f")
            nc.gpsimd.memset(ones_f[:], 1.0)
            nc.vector.tensor_copy(ones_sb[:], ones_f[:])
            # HAM warmup: keep the PE busy while the first weight/ht DMAs land
            for _ in range(32):
                wps = cycp.tile([128, 128], F32, name="wps", tag="cyc")
                nc.tensor.matmul(wps[:], ones_sb[:], ones_sb[:], start=True, stop=True)
            # ---- ht halves: [128, 16, 512] each
            def load_ht_half(c, half, pieces=1):
                # pieces>1: finer subtile gating so the first matmuls start early
                htt = htp.tile([128, NKO // 2, QC], BF16, name="ht", tag="ht")
                np_ = 16 // pieces
                for pi in range(pieces):
                    nc.sync.dma_start(
                        htt[:, pi * np_:(pi + 1) * np_, :],
                        hidden_p[c][:, half * 16 + pi * np_:half * 16 + (pi + 1) * np_, :],
                    )
                return htt

            # Startup DMA order matters: chunk-0 hidden + first weight slices
            # interleaved first so the first qkv round isn't queued behind 10MB
            # of weight loads; wo (needed ~60us in) goes last.
            w_sb = wp.tile([128, NKO, NJ * HD], BF16, name="w_sb")
            ht0_a = htp.tile([128, NKO // 2, QC], BF16, name="ht", tag="ht")
            for pi in range(4):
                nc.sync.dma_start(
                    ht0_a[:, pi * 4:(pi + 1) * 4, :], hidden_p[0][:, pi * 4:(pi + 1) * 4, :]
                )
                nc.sync.dma_start(
                    w_sb[:, pi * 4:(pi + 1) * 4, :], w_prep[:, pi * 4:(pi + 1) * 4, :]
                )
            ht0_b = load_ht_half(0, 1, pieces=2)
            for wi in range(4, 8):
                nc.sync.dma_start(
                    w_sb[:, wi * 4:(wi + 1) * 4, :], w_prep[:, wi * 4:(wi + 1) * 4, :]
                )
            nc.sync.dma_start(csa_sb[:], cs_a[:])
            nc.sync.dma_start(csb_sb[:], cs_b[:])
            nc.sync.dma_start(dmask_sb[:], dmask[:])
            wo_sb = wp.tile([128, HPC, HIDDEN], BF16, name="wo_sb")
            for wi in range(2):
                nc.sync.dma_start(
                    wo_sb[:, wi * 2:(wi + 1) * 2, :], wo_prep[:, wi * 2:(wi + 1) * 2, :]
                )
            ht_halves = [ht0_a, ht0_b]

            # ---- persistent K^T / V tiles (whole sequence, bf16)
            kT_tiles = []
            v_tiles = []
            for i in range(NTC):
                kt_i = kvp.tile([128, QC], BF16, name=f"kT{i}", tag=f"kT{i}")
                v_i = kvp.tile([128, 4, 128], BF16, name=f"v{i}", tag=f"v{i}")
                kT_tiles.append(kt_i)
                v_tiles.append(v_i)

            def rope6(dst):
                # neox partial rope on dst[0:64, :]: x1' = x1*c - x2*s ; x2' = x2*c + x1*s
                x1, x2 = dst[:HALF, :], dst[HALF:RD, :]
                tsl = slice(None)
                t1 = ropep.tile([HALF, QC], BF16, name="r1", tag="r1")
                t2 = ropep.tile([HALF, QC], BF16, name="r2", tag="r2")
                t3 = ropep.tile([HALF, QC], BF16, name="r3", tag="r3")
                t4 = ropep.tile([HALF, QC], BF16, name="r4", tag="r4")
                nc.vector.tensor_mul(t1[:], x1, csa_c[:HALF, :])   # x1*cos
                nc.vector.tensor_mul(t4[:], x1, csb_c[:HALF, :])   # x1*sin
                nc.vector.tensor_mul(t2[:], x2, csa_c[HALF:, :])   # x2*sin
                nc.vector.tensor_sub(x1, t1[:], t2[:])
                nc.vector.tensor_mul(t3[:], x2, csb_c[HALF:, :])   # x2*cos
                nc.vector.tensor_add(x2, t3[:], t4[:])

            def make_oproj_emitters(tci, ao):
                # one closure per (ts, oc) psum group of chunk tci's o_proj.
                # Evictions go to DVE when consumed during a long attention
                # phase (ACT is exp-saturated there), else to ACT.
                ems = []
                for ts in range(QC // 128):
                    for oc in range(HIDDEN // QC):
                        def em(ts=ts, oc=oc, tci=tci, ao=ao):
                            tl = tci * 4 + ts
                            po = cycp.tile([128, QC], F32, name="po", tag="cyc")
                            for h in range(HPC):
                                nc.tensor.matmul(
                                    po[:],
                                    ao[:, h, ts * 128:(ts + 1) * 128],
                                    wo_sb[:, h, oc * QC:(oc + 1) * QC],
                                    start=(h == 0), stop=(h == HPC - 1),
                                )
                            ob = obp.tile([128, QC], BF16, name="ob", tag="ob")
                            if oc % 2 == 0:
                                nc.scalar.copy(ob[:], po[:])
                            else:
                                nc.vector.tensor_copy(ob[:], po[:])
                            nc.sync.dma_start(out_p[tl, oc], ob[:])
                        ems.append(em)
                return ems

            op_ems = []   # deferred o_proj of the previous chunk
            for tci in range(NTC):
                csa_c = csa_sb[:, tci * QC:(tci + 1) * QC]
                csb_c = csb_sb[:, tci * QC:(tci + 1) * QC]

                # ================= phase 1: qkv^T, j-pairs with ko-inner =========
                qcur = qtp.tile([128, HPC, QC], BF16, name="qcur", tag="qt")
                vt = vtp.tile([128, QC], BF16, name="vt", tag="vt")
                ha, hb = ht_halves

                def qkv_round(ja, jb):
                    ps_a = cycp.tile([128, QC], F32, name="psa", tag="cyc")
                    ps_b = cycp.tile([128, QC], F32, name="psb", tag="cyc")
                    for ko in range(NKO):
                        htk = (ha if ko < 16 else hb)[:, ko % 16, :]
                        nc.tensor.matmul(
                            ps_a[:], w_sb[:, ko, ja * HD:(ja + 1) * HD], htk,
                            start=(ko == 0), stop=(ko == NKO - 1),
                        )
                        nc.tensor.matmul(
                            ps_b[:], w_sb[:, ko, jb * HD:(jb + 1) * HD], htk,
                            start=(ko == 0), stop=(ko == NKO - 1),
                        )
                    # evict each bank via ACT+DVE half-copies in parallel so the
                    # next round's matmuls get their psum banks back sooner
                    for idx, (j, ps) in enumerate(((ja, ps_a), (jb, ps_b))):
                        if j == HPC + 1:
                            dst = vt[:]
                        elif j == HPC:
                            dst = kT_tiles[tci][:]
                        else:
                            dst = qcur[:, j, :]
                        lo, hi = slice(0, QC // 2), slice(QC // 2, QC)
                        a, b = (lo, hi) if idx == 0 else (hi, lo)
                        nc.scalar.copy(dst[:, a], ps[:, a])
                        nc.vector.tensor_copy(dst[:, b], ps[:, b])
                        if j == HPC:
                            rope6(kT_tiles[tci])
                        elif j != HPC + 1:
                            rope6(qcur[:, j, :])

                qkv_round(HPC + 1, HPC)   # v, k first
                qkv_round(0, 1)
                qkv_round(2, 3)

                # prefetch next chunk's hidden (half A now, half B mid-attention)
                if tci + 1 < NTC:
                    ht_next_a = load_ht_half(tci + 1, 0)

                # v transpose via the DMA crossbar (no PE/psum involvement)
                nc.sync.dma_start_transpose(v_tiles[tci][:], vt[:])

                # ================= phase 2: attention, 2-head passes, kt-major ===
                # denominators via quad-grouped ex sums (DVE) + one ones-matmul
                # per quad; o_proj of the previous chunk interleaved as filler.
                nkt = 4 * tci + 4
                ngrp = nkt // 4
                kt_steps = 2 * nkt
                step = [0]
                op_i = [0]

                def maybe_fill():
                    step[0] += 1
                    want = (len(op_ems) * step[0]) // kt_steps
                    while op_i[0] < want:
                        op_ems[op_i[0]]()
                        op_i[0] += 1

                ao = aop.tile([128, HPC, QC], BF16, name="ao", tag="ao")

                def attn_pass(pas, qcur=qcur, ao=ao, tci=tci, nkt=nkt, ngrp=ngrp,
                              maybe_fill=maybe_fill):
                    h0 = 2 * pas
                    av0 = avp.tile([128, QC], F32, name="av0", tag="av")
                    av1 = avp.tile([128, QC], F32, name="av1", tag="av")
                    avs = (av0, av1)
                    dn0 = dnp.tile([128, QC], F32, name="dn0", tag="dn")
                    dn1 = dnp.tile([128, QC], F32, name="dn1", tag="dn")
                    dns = (dn0, dn1)

                    def emit_av(pkt, pexs):
                        _po = pkt - 4 * tci
                        pqs = slice(0 if _po < 0 else min(_po * 128, QC - 256), QC)
                        for hh in range(2):
                            nc.tensor.matmul(
                                avs[hh][:, pqs], v_tiles[pkt >> 2][:, pkt & 3, :],
                                pexs[hh][:, pqs],
                                start=(pkt == 0), stop=(pkt == nkt - 1),
                            )

                    def emit_dn(gi, qtiles):
                        for hh in range(2):
                            nc.tensor.matmul(
                                dns[hh][:], ones_sb[:], qtiles[hh][:],
                                start=(gi == 0), stop=(gi == ngrp - 1),
                            )

                    def emit_quad(grp_ex):
                        qtiles = []
                        for hh in range(2):
                            a, b, c, d = grp_ex[hh]
                            s1 = exqp.tile([128, QC], BF16, name="exq1", tag="exq1")
                            s2 = exqp.tile([128, QC], BF16, name="exq2", tag="exq2")
                            nc.vector.tensor_add(s1[:], a[:], b[:])
                            nc.vector.tensor_add(s2[:], c[:], d[:])
                            nc.vector.tensor_add(s1[:], s1[:], s2[:])
                            qtiles.append(s1)
                        return qtiles

                    pend_av = []
                    grp_ex = [[], []]
                    pend_dn = []
                    for kt in range(nkt):
                        _o = kt - 4 * tci
                        qoff = 0 if _o < 0 else min(_o * 128, QC - 256)
                        qs = slice(qoff, QC)
                        kT_l = kT_tiles[kt >> 2][:, (kt & 3) * 128:((kt & 3) + 1) * 128]
                        exs = []
                        for hh in range(2):
                            ss = ssp.tile([128, QC], F32, name="ss", tag="ss")
                            nc.tensor.matmul(
                                ss[:, qs], kT_l, qcur[:, h0 + hh, qs],
                                start=True, stop=True,
                            )
                            # full-width exp: [0:qoff) holds stale-but-finite
                            # scores; the causal mask below zeroes that region.
                            ex = exp_pool.tile([128, QC], BF16, name="ex", tag="ex")
                            nc.scalar.activation(ex[:], ss[:], EXP)
                            if _o >= 0:
                                _off = _o * 128
                                nc.vector.tensor_mul(
                                    ex[:], ex[:], dmask_sb[:, 384 - _off:896 - _off],
                                )
                            exs.append(ex)
                            grp_ex[hh].append(ex)
                        if len(grp_ex[0]) == 4:
                            pend_dn.append((kt // 4, emit_quad(grp_ex)))
                            grp_ex = [[], []]
                        pend_av.append((kt, exs))
                        if len(pend_av) > 2:      # lag-2: DVE mask/quad slack
                            emit_av(*pend_av.pop(0))
                        while pend_dn and kt >= 4 * (pend_dn[0][0] + 1) + 1:
                            emit_dn(*pend_dn.pop(0))
                        maybe_fill()
                    for pa in pend_av:
                        emit_av(*pa)
                    for g in pend_dn:
                        emit_dn(*g)
                    # normalize + evict: ao[:,h,:] = av * (1/denom); denom is
                    # already replicated across partitions by the ones-matmul
                    for hh in range(2):
                        rd_sb = nrmp.tile([128, QC], F32, name="rd", tag="rd")
                        nc.vector.reciprocal_approx_fast(rd_sb[:], dns[hh][:])
                        nc.vector.tensor_mul(ao[:, h0 + hh, :], avs[hh][:], rd_sb[:])
                        if DEBUG:
                            dnc = nrmp.tile([1, QC], F32, name="dnc", tag="dnc")
                            nc.scalar.copy(dnc[:], dns[hh][0:1, :])
                            nc.sync.dma_start(dbg_dn[tci, pas, hh], dnc[:])

                attn_pass(0)
                if tci + 1 < NTC:
                    ht_next_b = load_ht_half(tci + 1, 1)
                attn_pass(1)
                # flush any leftover o_proj of the previous chunk
                while op_i[0] < len(op_ems):
                    op_ems[op_i[0]]()
                    op_i[0] += 1
                op_ems = make_oproj_emitters(tci, ao)
                if DEBUG:
                    nc.sync.dma_start(dbg_q[tci], qcur[:])
                    nc.sync.dma_start(dbg_k[tci], kT_tiles[tci][:])
                    nc.sync.dma_start(dbg_v[tci], v_tiles[tci][:])
                    nc.sync.dma_start(dbg_ao[tci], ao[:])

                if tci + 1 < NTC:
                    ht_halves = [ht_next_a, ht_next_b]

            # o_proj of the final chunk
            for em in op_ems:
                em()
    nc.compile()
    return nc


def _host_prep(positions, hidden_states, w_qkv, w_o):
    import ml_dtypes
    BF = ml_dtypes.bfloat16

    positions = np.asarray(positions)
    hidden_states = np.asarray(hidden_states, dtype=np.float32)
    w_qkv = np.asarray(w_qkv, dtype=np.float32)
    w_o = np.asarray(w_o, dtype=np.float32)

    # hidden_p[c, p, ko, t] = hidden[c*QC + t, ko*128 + p]
    hidden_p = np.ascontiguousarray(
        hidden_states.reshape(NTC, QC, NKO, 128).transpose(0, 3, 2, 1).astype(BF)
    )

    pos = positions.astype(np.float32)
    r = np.arange(0, RD, 2, dtype=np.float32) / np.float32(RD)
    inv_freq = (np.float32(1.0) / (np.float32(ROPE_BASE) ** r)).astype(np.float32)
    ang = pos[:, None] * inv_freq[None, :]
    cos_t = np.cos(ang).astype(np.float32).T       # [32, T]
    sin_t = np.sin(ang).astype(np.float32).T
    cs_a = np.ascontiguousarray(np.concatenate([cos_t, sin_t], 0).astype(BF))
    cs_b = np.ascontiguousarray(np.concatenate([sin_t, cos_t], 0).astype(BF))

    p = np.arange(128, dtype=np.int64)[:, None]
    x = np.arange(896, dtype=np.int64)[None, :]
    dmask = np.ascontiguousarray((x >= p + 384).astype(BF))  # [128, 896]

    scale = np.float32(HD ** -0.5)
    q_size = NH * HD
    kv_size = NKV * HD
    in_maps = []
    for c in range(NC_CORES):
        wq = w_qkv[:, c * HPC * HD:(c + 1) * HPC * HD] * scale
        wk = w_qkv[:, q_size + c * HD:q_size + (c + 1) * HD]
        wv = w_qkv[:, q_size + kv_size + c * HD:q_size + kv_size + (c + 1) * HD]
        w_cat = np.concatenate([wq, wk, wv], axis=1)          # [4096, 768]
        # w_prep[p, ko, j] = w_cat[ko*128 + p, j]
        w_prep = np.ascontiguousarray(
            w_cat.reshape(NKO, 128, NJ * HD).transpose(1, 0, 2).astype(BF)
        )
        # wo_prep[d, h, o] = w_o[(c*HPC + h)*128 + d, o]
        wo_blk = w_o[c * HPC * HD:(c + 1) * HPC * HD, :]
        wo_prep = np.ascontiguousarray(
            wo_blk.reshape(HPC, 128, HIDDEN).transpose(1, 0, 2).astype(BF)
        )
        in_maps.append(
            {
                "hidden_p": hidden_p,
                "w_prep": w_prep,
                "wo_prep": wo_prep,
                "cs_a": cs_a,
                "cs_b": cs_b,
                "dmask": dmask,
            }
        )
    return in_maps


def kernel(positions, hidden_states, w_qkv, w_o, _trace=False, _trace_kw=None):
    from concourse.bass_utils import run_bass_kernel_spmd

    key = f"nc_dbg{DEBUG}"
    if key not in _CACHE:
        _CACHE[key] = _build_nc()
    nc = _CACHE[key]

    in_maps = _host_prep(positions, hidden_states, w_qkv, w_o)
    kw = dict(_trace_kw or {})
    res = run_bass_kernel_spmd(
        nc, in_maps, list(range(NC_CORES)), trace=_trace, **kw
    )
    out = np.zeros((T, HIDDEN), np.float32)
    for c in range(NC_CORES):
        o = np.asarray(res.results[c]["out_p"]).astype(np.float32)
        # [32 tl, 8 oc, 128, 512] -> [4096, 4096]
        out += o.transpose(0, 2, 1, 3).reshape(T, HIDDEN)
    if _trace:
        _CACHE["last_exec_time_ns"] = res.exec_time_ns
        _CACHE["last_results"] = res
    return out


# revision 46
# speedup vs baseline: 1.0274x; 1.0169x over previous
"""Trainium2 Bass kernel for MiniMax softmax attention (T=4096, H=4096, 32 q heads,
8 kv heads, head_dim=128, partial neox RoPE, causal softmax, o_proj).

Sharding: tensor-parallel over heads across 8 NeuronCores. Core c computes q heads
4c..4c+3 (= kv-head group c): qkv^T projection -> RoPE -> causal attention ->
partial o_proj with its w_o row-block. Host sums the 8 partial outputs.

v2 design (vs v1 baseline at ~1210us):
  - all matmul operands bf16 (PSUM accumulates fp32); halves DMA + enables DVE 2x
  - o_proj fused per t-chunk (no DRAM spill of attention outputs)
  - attention kt-major in 2-head passes; softmax denominator ones-matmuls packed
    2-at-a-time into PE column groups via tile_position (M=1 matmuls cost full
    stream time otherwise: 143us of PE in v1)
  - lag-1 software pipelining: AV/denominator matmuls for key-tile kt are emitted
    after the scores matmuls of kt+1 so the tensor FIFO never blocks on ACT exp
  - host pre-tiles every DRAM operand into [128, ...] partition-major contiguous
    blocks for wide DMA lines
PSUM budget: qkv/o_proj cycle pool 2 + scores 2 + AV 2 + denom 2 = 8 banks.
"""
import numpy as np

DEBUG = False

T = 4096
HIDDEN = 4096
NH = 32
NKV = 8
HD = 128
RD = 64
HALF = 32
ROPE_BASE = 10000000.0
NC_CORES = 8
HPC = NH // NC_CORES      # 4 q heads per core
QC = 512                  # t-chunk (query chunk)
NTC = T // QC             # 8 t-chunks
NKO = 32                  # hidden contraction tiles of 128
NJ = HPC + 2              # 6 qkv output tiles of 128 per core (q0..q3, k, v)

_CACHE = {}


def _build_nc():
    import concourse.mybir as mybir
    import concourse.tile as tile
    from concourse import bacc

    F32 = mybir.dt.float32
    BF16 = mybir.dt.bfloat16
    EXP = mybir.ActivationFunctionType.Exp

    nc = bacc.Bacc()
    hidden_p = nc.dram_tensor("hidden_p", [NTC, 128, NKO, QC], BF16, kind="ExternalInput")
    w_prep = nc.dram_tensor("w_prep", [128, NKO, NJ * HD], BF16, kind="ExternalInput")
    wo_prep = nc.dram_tensor("wo_prep", [128, HPC, HIDDEN], BF16, kind="ExternalInput")
    cs_a = nc.dram_tensor("cs_a", [RD, T], BF16, kind="ExternalInput")   # [cos;sin]
    cs_b = nc.dram_tensor("cs_b", [RD, T], BF16, kind="ExternalInput")   # [sin;cos]
    dmask = nc.dram_tensor("dmask", [128, 896], BF16, kind="ExternalInput")
    # out_p[tl, oc] = rows tl*128..tl*128+127, cols oc*512..oc*512+511
    out_p = nc.dram_tensor("out_p", [T // 128, HIDDEN // QC, 128, QC], BF16,
                           kind="ExternalOutput")
    if DEBUG:
        dbg_q = nc.dram_tensor("dbg_q", [NTC, 128, HPC, QC], BF16, kind="ExternalOutput")
        dbg_k = nc.dram_tensor("dbg_k", [NTC, 128, QC], BF16, kind="ExternalOutput")
        dbg_v = nc.dram_tensor("dbg_v", [NTC, 128, 4, 128], BF16, kind="ExternalOutput")
        dbg_dn = nc.dram_tensor("dbg_dn", [NTC, 2, 2, QC], F32, kind="ExternalOutput")
        dbg_ao = nc.dram_tensor("dbg_ao", [NTC, 128, HPC, QC], BF16, kind="ExternalOutput")

    with tile.TileContext(nc) as tc:
        with (
            tc.tile_pool(name="const", bufs=1) as const,
            tc.tile_pool(name="w", bufs=1) as wp,
            tc.tile_pool(name="kv", bufs=1) as kvp,
            tc.tile_pool(name="ht", bufs=3) as htp,
            tc.tile_pool(name="qt", bufs=1) as qtp,
            tc.tile_pool(name="rope", bufs=1) as ropep,
            tc.tile_pool(name="vt", bufs=1) as vtp,
            tc.tile_pool(name="ex", bufs=12) as exp_pool,
            tc.tile_pool(name="exq", bufs=2) as exqp,
            tc.tile_pool(name="ao", bufs=2) as aop,
            tc.tile_pool(name="nrm", bufs=2) as nrmp,
            tc.tile_pool(name="ob", bufs=4) as obp,
            tc.tile_pool(name="cyc", bufs=2, space="PSUM") as cycp,
            tc.tile_pool(name="ssp", bufs=2, space="PSUM") as ssp,
            tc.tile_pool(name="avp", bufs=2, space="PSUM") as avp,
            tc.tile_pool(name="dnp", bufs=2, space="PSUM") as dnp,
        ):
            # ---- constants
            csa_sb = const.tile([RD, T], BF16, name="csa", tag="csa")
            csb_sb = const.tile([RD, T], BF16, name="csb", tag="csb")
            dmask_sb = const.tile([128, 896], BF16, name="dmask", tag="dmask")
            # [128,128] ones: M=128 denominator matmuls cost the same as M=1 but
            # land the denominator pre-broadcast across all psum partitions
            ones_sb = const.tile([128, 128], BF16, name="ones", tag="ones")
            ones_f = const.tile([128, 128], F32, name="ones_f", tag="ones_f")
            nc.gpsimd.memset(ones_f[:], 1.0)
            nc.vector.tensor_copy(ones_sb[:], ones_f[:])
            # HAM warmup sized to span the ~12us startup DMA window so the PE
            # doesn't re-throttle before the first real matmuls
            wu = const.tile([128, QC], BF16, name="wu", tag="wu")
            nc.gpsimd.memset(wu[:], 0.0)
            for _ in range(48):
                wps = cycp.tile([128, QC], F32, name="wps", tag="cyc")
                nc.tensor.matmul(wps[:], ones_sb[:], wu[:], start=True, stop=True)
            # ---- ht halves: [128, 16, 512] each
            def load_ht_half(c, half, pieces=1):
                # pieces>1: finer subtile gating so the first matmuls start early
                htt = htp.tile([128, NKO // 2, QC], BF16, name="ht", tag="ht")
                np_ = 16 // pieces
                for pi in range(pieces):
                    nc.sync.dma_start(
                        htt[:, pi * np_:(pi + 1) * np_, :],
                        hidden_p[c][:, half * 16 + pi * np_:half * 16 + (pi + 1) * np_, :],
                    )
                return htt

            # Startup DMA order matters: chunk-0 hidden + first weight slices
            # interleaved first so the first qkv round isn't queued behind 10MB
            # of weight loads; wo (needed ~60us in) goes last.
            w_sb = wp.tile([128, NKO, NJ * HD], BF16, name="w_sb")
            ht0_a = htp.tile([128, NKO // 2, QC], BF16, name="ht", tag="ht")
            for pi in range(4):
                nc.sync.dma_start(
                    ht0_a[:, pi * 4:(pi + 1) * 4, :], hidden_p[0][:, pi * 4:(pi + 1) * 4, :]
                )
                nc.sync.dma_start(
                    w_sb[:, pi * 4:(pi + 1) * 4, :], w_prep[:, pi * 4:(pi + 1) * 4, :]
                )
            ht0_b = load_ht_half(0, 1, pieces=2)
            for wi in range(4, 8):
                nc.sync.dma_start(
                    w_sb[:, wi * 4:(wi + 1) * 4, :], w_prep[:, wi * 4:(wi + 1) * 4, :]
                )
            nc.sync.dma_start(csa_sb[:], cs_a[:])
            nc.sync.dma_start(csb_sb[:], cs_b[:])
            nc.sync.dma_start(dmask_sb[:], dmask[:])
            wo_sb = wp.tile([128, HPC, HIDDEN], BF16, name="wo_sb")
            for wi in range(2):
                nc.sync.dma_start(
                    wo_sb[:, wi * 2:(wi + 1) * 2, :], wo_prep[:, wi * 2:(wi + 1) * 2, :]
                )
            ht_halves = [ht0_a, ht0_b]

            # ---- persistent K^T / V tiles (whole sequence, bf16)
            kT_tiles = []
            v_tiles = []
            for i in range(NTC):
                kt_i = kvp.tile([128, QC], BF16, name=f"kT{i}", tag=f"kT{i}")
                v_i = kvp.tile([128, 4, 128], BF16, name=f"v{i}", tag=f"v{i}")
                kT_tiles.append(kt_i)
                v_tiles.append(v_i)

            def rope6(dst):
                # neox partial rope on dst[0:64, :]: x1' = x1*c - x2*s ; x2' = x2*c + x1*s
                x1, x2 = dst[:HALF, :], dst[HALF:RD, :]
                tsl = slice(None)
                t1 = ropep.tile([HALF, QC], BF16, name="r1", tag="r1")
                t2 = ropep.tile([HALF, QC], BF16, name="r2", tag="r2")
                t3 = ropep.tile([HALF, QC], BF16, name="r3", tag="r3")
                t4 = ropep.tile([HALF, QC], BF16, name="r4", tag="r4")
                nc.vector.tensor_mul(t1[:], x1, csa_c[:HALF, :])   # x1*cos
                nc.vector.tensor_mul(t4[:], x1, csb_c[:HALF, :])   # x1*sin
                nc.vector.tensor_mul(t2[:], x2, csa_c[HALF:, :])   # x2*sin
                nc.vector.tensor_sub(x1, t1[:], t2[:])
                nc.vector.tensor_mul(t3[:], x2, csb_c[HALF:, :])   # x2*cos
                nc.vector.tensor_add(x2, t3[:], t4[:])

            def make_oproj_emitters(tci, ao):
                # one closure per (ts, oc) psum group of chunk tci's o_proj.
                # Evictions go to DVE when consumed during a long attention
                # phase (ACT is exp-saturated there), else to ACT.
                ems = []
                for ts in range(QC // 128):
                    for oc in range(HIDDEN // QC):
                        def em(ts=ts, oc=oc, tci=tci, ao=ao):
                            tl = tci * 4 + ts
                            po = cycp.tile([128, QC], F32, name="po", tag="cyc")
                            for h in range(HPC):
                                nc.tensor.matmul(
                                    po[:],
                                    ao[:, h, ts * 128:(ts + 1) * 128],
                                    wo_sb[:, h, oc * QC:(oc + 1) * QC],
                                    start=(h == 0), stop=(h == HPC - 1),
                                )
                            ob = obp.tile([128, QC], BF16, name="ob", tag="ob")
                            if oc % 2 == 0:
                                nc.scalar.copy(ob[:], po[:])
                            else:
                                nc.vector.tensor_copy(ob[:], po[:])
                            nc.sync.dma_start(out_p[tl, oc], ob[:])
                        ems.append(em)
                return ems

            op_ems = []   # deferred o_proj of the previous chunk
            for tci in range(NTC):
                csa_c = csa_sb[:, tci * QC:(tci + 1) * QC]
                csb_c = csb_sb[:, tci * QC:(tci + 1) * QC]

                # ================= phase 1: qkv^T, j-pairs with ko-inner =========
                qcur = qtp.tile([128, HPC, QC], BF16, name="qcur", tag="qt")
                vt = vtp.tile([128, QC], BF16, name="vt", tag="vt")
                ha, hb = ht_halves

                def qkv_round(ja, jb, pool, tag):
                    # rounds alternate cyc/ss pools so each round's psum banks
                    # were already freed a full round earlier (no boundary wait)
                    ps_a = pool.tile([128, QC], F32, name="psa", tag=tag)
                    ps_b = pool.tile([128, QC], F32, name="psb", tag=tag)
                    for ko in range(NKO):
                        htk = (ha if ko < 16 else hb)[:, ko % 16, :]
                        nc.tensor.matmul(
                            ps_a[:], w_sb[:, ko, ja * HD:(ja + 1) * HD], htk,
                            start=(ko == 0), stop=(ko == NKO - 1),
                        )
                        nc.tensor.matmul(
                            ps_b[:], w_sb[:, ko, jb * HD:(jb + 1) * HD], htk,
                            start=(ko == 0), stop=(ko == NKO - 1),
                        )
                    # evict each bank via ACT+DVE half-copies in parallel so the
                    # next round's matmuls get their psum banks back sooner
                    for idx, (j, ps) in enumerate(((ja, ps_a), (jb, ps_b))):
                        if j == HPC + 1:
                            dst = vt[:]
                        elif j == HPC:
                            dst = kT_tiles[tci][:]
                        else:
                            dst = qcur[:, j, :]
                        lo, hi = slice(0, QC // 2), slice(QC // 2, QC)
                        a, b = (lo, hi) if idx == 0 else (hi, lo)
                        nc.scalar.copy(dst[:, a], ps[:, a])
                        nc.vector.tensor_copy(dst[:, b], ps[:, b])
                        if j == HPC:
                            rope6(kT_tiles[tci])
                        elif j != HPC + 1:
                            rope6(qcur[:, j, :])

                qkv_round(HPC + 1, HPC, cycp, "cyc")   # v, k first
                qkv_round(0, 1, ssp, "ss")
                qkv_round(2, 3, cycp, "cyc")

                # prefetch next chunk's hidden (half A now, half B mid-attention)
                if tci + 1 < NTC:
                    ht_next_a = load_ht_half(tci + 1, 0)

                # v transpose via the DMA crossbar (no PE/psum involvement)
                nc.sync.dma_start_transpose(v_tiles[tci][:], vt[:])

                # ================= phase 2: attention, 2-head passes, kt-major ===
                # denominators via quad-grouped ex sums (DVE) + one ones-matmul
                # per quad; o_proj of the previous chunk interleaved as filler.
                nkt = 4 * tci + 4
                ngrp = nkt // 4
                kt_steps = 2 * nkt
                step = [0]
                op_i = [0]

                def maybe_fill():
                    step[0] += 1
                    want = (len(op_ems) * step[0]) // kt_steps
                    while op_i[0] < want:
                        op_ems[op_i[0]]()
                        op_i[0] += 1

                ao = aop.tile([128, HPC, QC], BF16, name="ao", tag="ao")

                def attn_pass(pas, qcur=qcur, ao=ao, tci=tci, nkt=nkt, ngrp=ngrp,
                              maybe_fill=maybe_fill):
                    h0 = 2 * pas
                    av0 = avp.tile([128, QC], F32, name="av0", tag="av")
                    av1 = avp.tile([128, QC], F32, name="av1", tag="av")
                    avs = (av0, av1)
                    dn0 = dnp.tile([128, QC], F32, name="dn0", tag="dn")
                    dn1 = dnp.tile([128, QC], F32, name="dn1", tag="dn")
                    dns = (dn0, dn1)

                    def emit_av(pkt, pexs):
                        _po = pkt - 4 * tci
                        pqs = slice(0 if _po < 0 else min(_po * 128, QC - 256), QC)
                        for hh in range(2):
                            nc.tensor.matmul(
                                avs[hh][:, pqs], v_tiles[pkt >> 2][:, pkt & 3, :],
                                pexs[hh][:, pqs],
                                start=(pkt == 0), stop=(pkt == nkt - 1),
                            )

                    def emit_dn(gi, qtiles):
                        for hh in range(2):
                            nc.tensor.matmul(
                                dns[hh][:], ones_sb[:], qtiles[hh][:],
                                start=(gi == 0), stop=(gi == ngrp - 1),
                            )

                    def emit_quad(grp_ex):
                        qtiles = []
                        for hh in range(2):
                            a, b, c, d = grp_ex[hh]
                            s1 = exqp.tile([128, QC], BF16, name="exq1", tag="exq1")
                            s2 = exqp.tile([128, QC], BF16, name="exq2", tag="exq2")
                            nc.vector.tensor_add(s1[:], a[:], b[:])
                            nc.vector.tensor_add(s2[:], c[:], d[:])
                            nc.vector.tensor_add(s1[:], s1[:], s2[:])
                            qtiles.append(s1)
                        return qtiles

                    pend_av = []
                    grp_ex = [[], []]
                    pend_dn = []
                    for kt in range(nkt):
                        _o = kt - 4 * tci
                        qoff = 0 if _o < 0 else min(_o * 128, QC - 256)
                        qs = slice(qoff, QC)
                        kT_l = kT_tiles[kt >> 2][:, (kt & 3) * 128:((kt & 3) + 1) * 128]
                        exs = []
                        for hh in range(2):
                            ss = ssp.tile([128, QC], F32, name="ss", tag="ss")
                            nc.tensor.matmul(
                                ss[:, qs], kT_l, qcur[:, h0 + hh, qs],
                                start=True, stop=True,
                            )
                            # full-width exp: [0:qoff) holds stale-but-finite
                            # scores; the causal mask below zeroes that region.
                            ex = exp_pool.tile([128, QC], BF16, name="ex", tag="ex")
                            nc.scalar.activation(ex[:], ss[:], EXP)
                            if _o >= 0:
                                _off = _o * 128
                                nc.vector.tensor_mul(
                                    ex[:], ex[:], dmask_sb[:, 384 - _off:896 - _off],
                                )
                            exs.append(ex)
                            grp_ex[hh].append(ex)
                        if len(grp_ex[0]) == 4:
                            pend_dn.append((kt // 4, emit_quad(grp_ex)))
                            grp_ex = [[], []]
                        pend_av.append((kt, exs))
                        if len(pend_av) > 2:      # lag-2: DVE mask/quad slack
                            emit_av(*pend_av.pop(0))
                        while pend_dn and kt >= 4 * (pend_dn[0][0] + 1) + 1:
                            emit_dn(*pend_dn.pop(0))
                        maybe_fill()
                    for pa in pend_av:
                        emit_av(*pa)
                    for g in pend_dn:
                        emit_dn(*g)
                    # normalize + evict: ao[:,h,:] = av * (1/denom); denom is
                    # already replicated across partitions by the ones-matmul
                    for hh in range(2):
                        rd_sb = nrmp.tile([128, QC], F32, name="rd", tag="rd")
                        nc.vector.reciprocal_approx_fast(rd_sb[:], dns[hh][:])
                        nc.vector.tensor_mul(ao[:, h0 + hh, :], avs[hh][:], rd_sb[:])
                        if DEBUG:
                            dnc = nrmp.tile([1, QC], F32, name="dnc", tag="dnc")
                            nc.scalar.copy(dnc[:], dns[hh][0:1, :])
                            nc.sync.dma_start(dbg_dn[tci, pas, hh], dnc[:])

                attn_pass(0)
                if tci + 1 < NTC:
                    ht_next_b = load_ht_half(tci + 1, 1)
                attn_pass(1)
                # flush any leftover o_proj of the previous chunk
                while op_i[0] < len(op_ems):
                    op_ems[op_i[0]]()
                    op_i[0] += 1
                op_ems = make_oproj_emitters(tci, ao)
                if DEBUG:
                    nc.sync.dma_start(dbg_q[tci], qcur[:])
                    nc.sync.dma_start(dbg_k[tci], kT_tiles[tci][:])
                    nc.sync.dma_start(dbg_v[tci], v_tiles[tci][:])
                    nc.sync.dma_start(dbg_ao[tci], ao[:])

                if tci + 1 < NTC:
                    ht_halves = [ht_next_a, ht_next_b]

            # o_proj of the final chunk
            for em in op_ems:
                em()
    nc.compile()
    return nc


def _host_prep(positions, hidden_states, w_qkv, w_o):
    import ml_dtypes
    BF = ml_dtypes.bfloat16

    positions = np.asarray(positions)
    hidden_states = np.asarray(hidden_states, dtype=np.float32)
    w_qkv = np.asarray(w_qkv, dtype=np.float32)
    w_o = np.asarray(w_o, dtype=np.float32)

    # hidden_p[c, p, ko, t] = hidden[c*QC + t, ko*128 + p]
    hidden_p = np.ascontiguousarray(
        hidden_states.reshape(NTC, QC, NKO, 128).transpose(0, 3, 2, 1).astype(BF)
    )

    pos = positions.astype(np.float32)
    r = np.arange(0, RD, 2, dtype=np.float32) / np.float32(RD)
    inv_freq = (np.float32(1.0) / (np.float32(ROPE_BASE) ** r)).astype(np.float32)
    ang = pos[:, None] * inv_freq[None, :]
    cos_t = np.cos(ang).astype(np.float32).T       # [32, T]
    sin_t = np.sin(ang).astype(np.float32).T
    cs_a = np.ascontiguousarray(np.concatenate([cos_t, sin_t], 0).astype(BF))
    cs_b = np.ascontiguousarray(np.concatenate([sin_t, cos_t], 0).astype(BF))

    p = np.arange(128, dtype=np.int64)[:, None]
    x = np.arange(896, dtype=np.int64)[None, :]
    dmask = np.ascontiguousarray((x >= p + 384).astype(BF))  # [128, 896]

    scale = np.float32(HD ** -0.5)
    q_size = NH * HD
    kv_size = NKV * HD
    in_maps = []
    for c in range(NC_CORES):
        wq = w_qkv[:, c * HPC * HD:(c + 1) * HPC * HD] * scale
        wk = w_qkv[:, q_size + c * HD:q_size + (c + 1) * HD]
        wv = w_qkv[:, q_size + kv_size + c * HD:q_size + kv_size + (c + 1) * HD]
        w_cat = np.concatenate([wq, wk, wv], axis=1)          # [4096, 768]
        # w_prep[p, ko, j] = w_cat[ko*128 + p, j]
        w_prep = np.ascontiguousarray(
            w_cat.reshape(NKO, 128, NJ * HD).transpose(1, 0, 2).astype(BF)
        )
        # wo_prep[d, h, o] = w_o[(c*HPC + h)*128 + d, o]
        wo_blk = w_o[c * HPC * HD:(c + 1) * HPC * HD, :]
        wo_prep = np.ascontiguousarray(
            wo_blk.reshape(HPC, 128, HIDDEN).transpose(1, 0, 2).astype(BF)
        )
        in_maps.append(
            {
                "hidden_p": hidden_p,
                "w_prep": w_prep,
                "wo_prep": wo_prep,
                "cs_a": cs_a,
                "cs_b": cs_b,
                "dmask": dmask,
            }
        )
    return in_maps


def kernel(positions, hidden_states, w_qkv, w_o, _trace=False, _trace_kw=None):
    from concourse.bass_utils import run_bass_kernel_spmd

    key = f"nc_dbg{DEBUG}"
    if key not in _CACHE:
        _CACHE[key] = _build_nc()
    nc = _CACHE[key]

    in_maps = _host_prep(positions, hidden_states, w_qkv, w_o)
    kw = dict(_trace_kw or {})
    res = run_bass_kernel_spmd(
        nc, in_maps, list(range(NC_CORES)), trace=_trace, **kw
    )
    out = np.zeros((T, HIDDEN), np.float32)
    for c in range(NC_CORES):
        o = np.asarray(res.results[c]["out_p"]).astype(np.float32)
        # [32 tl, 8 oc, 128, 512] -> [4096, 4096]
        out += o.transpose(0, 2, 1, 3).reshape(T, HIDDEN)
    if _trace:
        _CACHE["last_exec_time_ns"] = res.exec_time_ns
        _CACHE["last_results"] = res
    return out
